# revision 6
# baseline (speedup 1.0000x reference)
"""Trainium2 Bass kernel for nn_CausalPredictor_46462956208724.

Math: the reference computes
    wy = xm @ Wy_w.T + Wy_b            [L, 1]
    wz = dic_z @ Wz_w.T + Wz_b         [1, 1]
    attention = softmax(wy @ wz.T, axis=1)   # axis of size 1 -> exactly 1.0
    z = (attention * prior) @ dic_z    [L, C]
Softmax over a size-1 axis is exactly 1.0 in fp32 (exp(0)/exp(0)), so
    z[l, :] = prior[0] * dic[1, 0, :]   for every row l.
The output is a broadcast of one scaled 1024-float row to 131072 rows —
a pure HBM-write problem (512 MB of output).

Sharding: pure data parallel over rows. 8 cores x 16384 rows each; the
tiny row + prior are replicated. Each core scales the row on-chip,
replicates it across 128 SBUF partitions x 16 repeats (8 MB tile) and
issues 8 x 8 MB DMA stores to fill its 64 MB output shard.
"""

import sys

for _p in (
    "/root/.axon_site",
    "/root/.axon_site/_ro/trn_rl_repo",
    "/root/.axon_site/_ro/pypackages",
    "/opt/trn_rl_repo",
):
    if _p not in sys.path:
        sys.path.append(_p)

import numpy as np

L = 131072
C = 1024
N_CORES = 8
SHARD = L // N_CORES          # 16384 rows per core
P = 128                       # SBUF partitions
RP = 16                       # output rows per partition per DMA block
FREE = RP * C                 # 16384 f32 per partition = 64 KB
NBLK = SHARD // (P * RP)      # 8 DMA blocks of 8 MB per core

_CACHE = {}


def _build_bass():
    import concourse.bacc as bacc
    import concourse.tile as tile
    from concourse import mybir

    f32 = mybir.dt.float32
    # Bacc (not raw Bass): its compile() pipeline splits multi-sem waits
    # into event semaphores — TRN2 allows at most 1 wait per instruction,
    # and walrus rejects the raw IR with "Too many sync wait commands".
    nc = bacc.Bacc(None)
    row_in = nc.declare_dram_parameter("row", [1, C], f32, isOutput=False)
    prior_in = nc.declare_dram_parameter("prior", [1, 1], f32, isOutput=False)
    out = nc.declare_dram_parameter("out", [SHARD, C], f32, isOutput=True)

    with tile.TileContext(nc) as tc:
        with tc.tile_pool(name="pool", bufs=1) as pool:
            # Load with a stride-0 partition dim on the DRAM side so every
            # SBUF partition receives the same row/scalar. A [1, N] DMA gets
            # sprayed across all 16 DMA queues, and a consumer waiting on
            # that many queue semaphores overflows the HW sync-wait slots
            # ("Too many sync wait commands" in walrus codegen); a 128-
            # partition transfer is a single normal DMA.
            col = pool.tile([P, C], f32)
            prb = pool.tile([P, 1], f32)
            nc.sync.dma_start(out=col[:], in_=row_in[:].partition_broadcast(P))
            nc.sync.dma_start(out=prb[:], in_=prior_in[:].partition_broadcast(P))

            # Interpose a same-engine copy so the multiply carries a single
            # semaphore wait (engine FIFO orders it after the prb2 copy).
            prb2 = pool.tile([P, 1], f32)
            nc.vector.tensor_copy(prb2[:], prb[:])

            big = pool.tile([P, FREE], f32)
            nc.vector.tensor_mul(
                big[:, 0:C], col[:], prb2[:].broadcast_to([P, C])
            )
            w = C
            while w < FREE:
                n = min(w, FREE - w)
                nc.vector.tensor_copy(big[:, w : w + n], big[:, 0:n])
                w += n

            out_v = out[:].rearrange("(b p r) c -> b p (r c)", p=P, r=RP)
            for b in range(NBLK):
                nc.sync.dma_start(out=out_v[b], in_=big[:])
    nc.compile()
    return nc


def _get_nc():
    if "nc" not in _CACHE:
        _CACHE["nc"] = _build_bass()
    return _CACHE["nc"]


def kernel(x, xm, Wy_w, Wy_b, Wz_w, Wz_b, dic, prior, **_unused):
    from concourse.bass_utils import run_bass_kernel_spmd

    nc = _get_nc()
    row = np.ascontiguousarray(np.asarray(dic, dtype=np.float32)[1].reshape(1, C))
    pr = np.ascontiguousarray(np.asarray(prior, dtype=np.float32).reshape(1, 1))
    in_maps = [{"row": row, "prior": pr} for _ in range(N_CORES)]
    res = run_bass_kernel_spmd(nc, in_maps, list(range(N_CORES)))
    shards = [res.results[i]["out"] for i in range(N_CORES)]
    full = np.concatenate(shards, axis=0).reshape(L, 1, C)
    return full
